# revision 5
# baseline (speedup 1.0000x reference)
"""Trainium2 Bass kernel for nn_DGM_Layer (gnn_message_passing).

Computes, for x [8, 2048, 512], W [512, 512], b [512], temperature, threshold:
  h     = relu(x_flat @ W + b)                       -> x_out [8, 2048, 512]
  xn    = scale * (h - centroid)   (centroid/scale are global over x_flat)
  Dmat  = ||xn_i||^2 + ||xn_j||^2 - 2 xn_i.xn_j      per batch graph
  A     = sigmoid(T * (thr - Dmat)), diag zeroed     [8, 2048, 2048] f32
  mask  = A > 0.5                                    [8, 2048, 2048] bool

Sharding: pure data parallel, one batch graph per NeuronCore (B=8 = n_cores).
Host does the cheap global reductions (centroid/scale), the x transpose
(device wants x^T as the matmul operand), and diagonal zeroing.
Matmuls run as float32r (hardware-rounded fp32): measured |dD| <= 1.5e-3 vs
an off-diagonal decision margin of 5.2, and ~2e-5 L2 error on G/h.
"""

import sys

if "/opt/trn_rl_repo" not in sys.path:
    sys.path.insert(0, "/opt/trn_rl_repo")

import numpy as np

B, N, D = 8, 2048, 512
P = 128
KT = D // P           # 4 K tiles of 128 (feature dim)
MT = N // P           # 16 row tiles of 128
NC = N // 512         # 4 column chunks of 512

_CACHE = {}


def _build_program():
    import concourse.bacc as bacc
    import concourse.mybir as mybir
    from concourse import tile

    f32 = mybir.dt.float32
    f32r = mybir.dt.float32r
    u8 = mybir.dt.uint8
    AF = mybir.ActivationFunctionType
    OP = mybir.AluOpType

    nc = bacc.Bacc(None, target_bir_lowering=False, debug=False)

    with tile.TileContext(nc) as tc:
        with tc.tile_pool(name="dram", bufs=1, space="DRAM") as dram:
            # inputs (per core)
            xT_in = dram.tile([P, KT, N], f32, kind="ExternalInput")     # x^T: [p, k, row] = x[row, k*128+p]
            W_in = dram.tile([P, KT, D], f32, kind="ExternalInput")      # [p, k, o] = W[k*128+p, o]
            b_in = dram.tile([P, KT], f32, kind="ExternalInput")         # [p, m] = b[m*128+p]
            sc_in = dram.tile([P, KT], f32, kind="ExternalInput")        # scale*centroid, same layout as b
            nsc_in = dram.tile([P, KT], f32, kind="ExternalInput")       # -scale*centroid
            scol_in = dram.tile([P, 4], f32, kind="ExternalInput")       # columns: [s, T, -T, thr]
            # outputs (per core)
            hT_out = dram.tile([P, KT, N], f32, kind="ExternalOutput")   # h^T, host transposes back
            A_out = dram.tile([N, N], f32, kind="ExternalOutput")
            mask_out = dram.tile([N, N], u8, kind="ExternalOutput")

            with (
                tc.tile_pool(name="consts", bufs=1) as consts,
                tc.tile_pool(name="xt_stage", bufs=2) as xt_stage,
                tc.tile_pool(name="xt_r", bufs=1) as xt_r_pool,
                tc.tile_pool(name="w_stage", bufs=2) as w_stage,
                tc.tile_pool(name="w_r", bufs=1) as w_r_pool,
                tc.tile_pool(name="ht", bufs=1) as ht_pool,
                tc.tile_pool(name="xnt", bufs=1) as xnt_pool,
                tc.tile_pool(name="xnt2", bufs=4) as xnt2_pool,
                tc.tile_pool(name="wb", bufs=1) as wb_pool,
                tc.tile_pool(name="small", bufs=1) as small,
                tc.tile_pool(name="v1", bufs=4) as v1_pool,
                tc.tile_pool(name="a_st", bufs=4) as a_pool,
                tc.tile_pool(name="m_st", bufs=2) as m_pool,
                tc.tile_pool(name="ps_mm", bufs=4, space="PSUM") as ps_mm,
                tc.tile_pool(name="ps_sq", bufs=1, space="PSUM") as ps_sq,
            ):
                # ---- constants ----
                b4 = consts.tile([P, KT], f32)
                sc4 = consts.tile([P, KT], f32)
                nsc4 = consts.tile([P, KT], f32)
                scol = consts.tile([P, 4], f32)
                nc.sync.dma_start(out=b4[:], in_=b_in[:])
                nc.sync.dma_start(out=sc4[:], in_=sc_in[:])
                nc.sync.dma_start(out=nsc4[:], in_=nsc_in[:])
                nc.sync.dma_start(out=scol[:], in_=scol_in[:])
                s_col = scol[:, 0:1]
                T_col = scol[:, 1:2]
                nT_col = scol[:, 2:3]
                thr_col = scol[:, 3:4]

                ones_f = consts.tile([P, P], f32)
                ones_r = consts.tile([P, P], f32r)
                nc.vector.memset(ones_f[:], 1.0)
                nc.vector.tensor_copy(ones_r[:], ones_f[:])

                # ---- load W, round to fp32r ----
                w_r = w_r_pool.tile([P, KT, D], f32r)
                for k in range(KT):
                    wst = w_stage.tile([P, D], f32)
                    nc.sync.dma_start(out=wst[:], in_=W_in[:, k, :])
                    nc.vector.tensor_copy(w_r[:, k, :], wst[:])

                # ---- load x^T, round to fp32r ----
                xt_r = xt_r_pool.tile([P, KT, N], f32r)
                for k in range(KT):
                    xst = xt_stage.tile([P, N], f32)
                    nc.sync.dma_start(out=xst[:], in_=xT_in[:, k, :])
                    nc.vector.tensor_copy(xt_r[:, k, :], xst[:])

                # ---- h^T = relu(W^T x^T + b): [feat_out on partitions, rows free] ----
                ht = ht_pool.tile([P, KT, N], f32)
                for m in range(KT):
                    for n in range(NC):
                        g = ps_mm.tile([P, 512], f32, tag="mm")
                        for k in range(KT):
                            nc.tensor.matmul(
                                g[:],
                                w_r[:, k, m * P:(m + 1) * P],
                                xt_r[:, k, n * 512:(n + 1) * 512],
                                start=(k == 0),
                                stop=(k == KT - 1),
                            )
                        nc.scalar.activation(
                            ht[:, m, n * 512:(n + 1) * 512], g[:], AF.Relu,
                            bias=b4[:, m:m + 1],
                        )
                    # DMA h^T out as soon as each m-stripe is done
                    nc.sync.dma_start(out=hT_out[:, m, :], in_=ht[:, m, :])

                # ---- xn^T (fp32r) and (xn^T)^2 (fp32r) ----
                xnt = xnt_pool.tile([P, KT, N], f32r)
                xnt2 = [None] * KT
                for m in range(KT):
                    nc.vector.tensor_scalar(
                        xnt[:, m, :], ht[:, m, :], s_col, sc4[:, m:m + 1],
                        OP.mult, OP.subtract,
                    )
                    t2 = xnt2_pool.tile([P, N], f32r, tag="xnt2")
                    nc.scalar.activation(
                        t2[:], ht[:, m, :], AF.Square,
                        bias=nsc4[:, m:m + 1], scale=s_col,
                    )
                    xnt2[m] = t2

                # ---- wb[p, j] = sq_j  (broadcast over partitions) ----
                wb = wb_pool.tile([P, N], f32)
                for c in range(NC):
                    wps = ps_mm.tile([P, 512], f32, tag="mm")
                    for m in range(KT):
                        nc.tensor.matmul(
                            wps[:], ones_r[:], xnt2[m][:, c * 512:(c + 1) * 512],
                            start=(m == 0), stop=(m == KT - 1),
                        )
                    nc.vector.tensor_copy(wb[:, c * 512:(c + 1) * 512], wps[:])

                # ---- sq per-partition: sq_sb[p, t] = sq_{t*128+p} ----
                # (fp32r matmul needs an even dst free dim -> N=2, keep col 0)
                sqps = ps_sq.tile([P, 2 * MT], f32)
                for t in range(MT):
                    for m in range(KT):
                        nc.tensor.matmul(
                            sqps[:, 2 * t:2 * t + 2],
                            xnt2[m][:, t * P:(t + 1) * P],
                            ones_r[:, 0:2],
                            start=(m == 0), stop=(m == KT - 1),
                        )
                sq_sb = small.tile([P, MT], f32)
                nc.vector.tensor_copy(sq_sb[:], sqps[:, 0:2 * MT:2])
                # m_i = sq - thr (mask compare);  beta_i = (sq - thr) * (-T) (sigmoid bias)
                mth = small.tile([P, MT], f32)
                beta = small.tile([P, MT], f32)
                nc.vector.tensor_scalar(mth[:], sq_sb[:], thr_col, None, OP.subtract)
                nc.vector.tensor_scalar(beta[:], sq_sb[:], thr_col, nT_col, OP.subtract, OP.mult)

                # ---- pairwise tiles ----
                for t in range(MT):
                    mstage = m_pool.tile([P, N], u8, tag="mst")
                    for c in range(NC):
                        g = ps_mm.tile([P, 512], f32, tag="mm")
                        for k in range(KT):
                            nc.tensor.matmul(
                                g[:],
                                xnt[:, k, t * P:(t + 1) * P],
                                xnt[:, k, c * 512:(c + 1) * 512],
                                start=(k == 0),
                                stop=(k == KT - 1),
                            )
                        # v1 = 2*G - sq_j
                        v1 = v1_pool.tile([P, 512], f32, tag="v1")
                        nc.vector.scalar_tensor_tensor(
                            v1[:], g[:], 2.0, wb[:, c * 512:(c + 1) * 512],
                            OP.mult, OP.subtract,
                        )
                        # A = sigmoid(T*v1 + T*(thr - sq_i))
                        a_st = a_pool.tile([P, 512], f32, tag="ast")
                        nc.scalar.activation(
                            a_st[:], v1[:], AF.Sigmoid,
                            bias=beta[:, t:t + 1], scale=T_col,
                        )
                        nc.sync.dma_start(
                            out=A_out[t * P:(t + 1) * P, c * 512:(c + 1) * 512],
                            in_=a_st[:],
                        )
                        # mask = v1 > (sq_i - thr)
                        nc.gpsimd.tensor_scalar(
                            mstage[:, c * 512:(c + 1) * 512], v1[:],
                            mth[:, t:t + 1], None, OP.is_gt,
                        )
                    nc.sync.dma_start(out=mask_out[t * P:(t + 1) * P, :], in_=mstage[:])

    nc.compile()
    names = dict(
        xT=xT_in.name, W=W_in.name, b=b_in.name, sc=sc_in.name,
        nsc=nsc_in.name, scol=scol_in.name,
        hT=hT_out.name, A=A_out.name, mask=mask_out.name,
    )
    return nc, names


def _get_exec():
    """Build (once) a jitted shard_map callable running the program on 8 cores.

    Mirrors concourse.bass2jax.run_bass_via_pjrt's multi-core path, but
    caches the callable so repeat calls don't re-trace, and lets us create
    the donated output buffers on-device.
    """
    if "exec" in _CACHE:
        return _CACHE["exec"]

    import jax
    import jax.numpy as jnp
    from jax.sharding import Mesh, NamedSharding, PartitionSpec
    from jax.experimental.shard_map import shard_map
    import concourse.mybir as mybir
    from concourse import bass2jax

    bass2jax.install_neuronx_cc_hook()
    nc, nm = _build_program()
    _CACHE["prog"] = (nc, nm)

    partition_name = nc.partition_id_tensor.name if nc.partition_id_tensor else None
    in_names, out_names, out_avals = [], [], []
    for alloc in nc.m.functions[0].allocations:
        if not isinstance(alloc, mybir.MemoryLocationSet):
            continue
        name = alloc.memorylocations[0].name
        if alloc.kind == "ExternalInput":
            if name != partition_name:
                in_names.append(name)
        elif alloc.kind == "ExternalOutput":
            out_names.append(name)
            out_avals.append(
                jax.core.ShapedArray(tuple(alloc.tensor_shape), mybir.dt.np(alloc.dtype))
            )
    n_params = len(in_names)
    n_outs = len(out_names)
    all_names = in_names + out_names
    if partition_name is not None:
        all_names = all_names + [partition_name]

    def _body(*args):
        operands = list(args)
        if partition_name is not None:
            operands.append(bass2jax.partition_id_tensor())
        outs = bass2jax._bass_exec_p.bind(
            *operands,
            out_avals=tuple(out_avals),
            in_names=tuple(all_names),
            out_names=tuple(out_names),
            lowering_input_output_aliases=(),
            sim_require_finite=True,
            sim_require_nnan=True,
            nc=nc,
        )
        return tuple(outs)

    devices = jax.devices()[:B]
    mesh = Mesh(np.asarray(devices), ("core",))
    spec = PartitionSpec("core")
    sharded = jax.jit(
        shard_map(
            _body, mesh=mesh,
            in_specs=(spec,) * (n_params + n_outs),
            out_specs=(spec,) * n_outs,
            check_rep=False,
        ),
        donate_argnums=tuple(range(n_params, n_params + n_outs)),
        keep_unused=True,
    )
    shardings = NamedSharding(mesh, spec)

    def zeros_fn():
        return tuple(
            jnp.zeros((B * a.shape[0], *a.shape[1:]), a.dtype) for a in out_avals
        )

    zfn = jax.jit(zeros_fn, out_shardings=(shardings,) * n_outs)

    ex = dict(
        nc=nc, nm=nm, in_names=in_names, out_names=out_names,
        out_avals=out_avals, sharded=sharded, zfn=zfn,
        sharding=shardings, mesh=mesh,
    )
    _CACHE["exec"] = ex
    return ex


def _device_inputs(ex, xT, W4, b4, sc4, nsc4, scol):
    import jax

    nm = ex["nm"]
    per_core = {
        nm["xT"]: xT,                                    # [B, P, KT, N]
        nm["W"]: np.broadcast_to(W4, (B, *W4.shape)),
        nm["b"]: np.broadcast_to(b4, (B, *b4.shape)),
        nm["sc"]: np.broadcast_to(sc4, (B, *sc4.shape)),
        nm["nsc"]: np.broadcast_to(nsc4, (B, *nsc4.shape)),
        nm["scol"]: np.broadcast_to(scol, (B, *scol.shape)),
    }
    arrs = []
    for name in ex["in_names"]:
        a = per_core[name]
        a = np.ascontiguousarray(a.reshape(a.shape[0] * a.shape[1], *a.shape[2:]))
        arrs.append(jax.device_put(a, ex["sharding"]))
    return arrs


def _host_prep(x, W, b, temperature, threshold):
    x = np.asarray(x, np.float32)
    W = np.asarray(W, np.float32)
    b = np.asarray(b, np.float32)
    T = np.float32(np.asarray(temperature).reshape(()))
    thr = np.float32(np.asarray(threshold).reshape(()))

    xf = x.reshape(-1, D)
    centroid = xf.mean(0, dtype=np.float32)
    scale = np.float32(0.9) / np.abs(xf - centroid).max()

    # x^T per core: [B, P, KT, N] with [p, k, row] = x[b, row, k*128+p]
    xT = np.ascontiguousarray(x.reshape(B, N, KT, P).transpose(0, 3, 2, 1))
    W4 = np.ascontiguousarray(W.reshape(KT, P, D).transpose(1, 0, 2))  # [P, KT, D]
    b4 = np.ascontiguousarray(b.reshape(KT, P).T)                      # [P, KT]
    sc4 = np.ascontiguousarray((scale * centroid).reshape(KT, P).T)
    nsc4 = np.ascontiguousarray((-scale * centroid).reshape(KT, P).T)
    scol = np.empty((P, 4), np.float32)
    scol[:, 0] = scale
    scol[:, 1] = T
    scol[:, 2] = -T
    scol[:, 3] = thr
    return xT, W4, b4, sc4, nsc4, scol


def kernel(x, W, b, temperature, threshold):
    import jax

    ex = _get_exec()
    prep = _host_prep(x, W, b, temperature, threshold)
    ins = _device_inputs(ex, *prep)
    outs = ex["sharded"](*ins, *ex["zfn"]())
    outs = jax.block_until_ready(outs)
    byname = dict(zip(ex["out_names"], outs))
    nm = ex["nm"]

    hT = np.asarray(byname[nm["hT"]]).reshape(B, P, KT, N)
    A = np.array(byname[nm["A"]]).reshape(B, N, N)
    mask = np.array(byname[nm["mask"]]).reshape(B, N, N)

    # h[row, m*128+p] = hT[p, m, row]
    x_out = np.ascontiguousarray(hT.transpose(0, 3, 2, 1).reshape(B, N, D))
    idx = np.arange(N)
    A[:, idx, idx] = 0.0
    mask[:, idx, idx] = 0
    return x_out, A, mask.view(bool)


def timed_run(inputs, reps=10):
    """Amortized per-iteration device execution time (ns). Chains donated
    output buffers between iterations so runs serialize on device; no host
    transfer of the 200MB outputs inside the loop."""
    import time
    import jax

    ex = _get_exec()
    prep = _host_prep(**inputs)
    ins = _device_inputs(ex, *prep)
    outs = ex["sharded"](*ins, *ex["zfn"]())  # warmup (compile)
    outs = jax.block_until_ready(outs)
    t0 = time.perf_counter()
    for _ in range(reps):
        outs = ex["sharded"](*ins, *outs)
    jax.block_until_ready(outs)
    t1 = time.perf_counter()
    return (t1 - t0) / reps * 1e9
